# revision 8
# baseline (speedup 1.0000x reference)
"""Single-head attention (B=4, S=2048, E=1024) on 8 TRN2 NeuronCores.

Sharding: data-parallel over (batch, query-half): core c handles batch c//2,
queries [h*1024, (h+1)*1024) where h = c%2. Each core computes K/V for its
full batch (duplicated within the pair) so there are no collectives.

Per-core layout trick: the host permutes the key/value sequence so that this
core's query block is always columns [0, 1024) of xT. Attention output is
invariant to a consistent permutation of keys+values, so K/V built from the
permuted xT give identical results.

Pipeline (all matmuls bf16 inputs, fp32 PSUM accumulation):
  KT[f,s]  = WkT.T @ xT          (bk dropped: constant-per-query shift,
                                  softmax-invariant)
  QT[f,sq] = WqT.T @ xT[:, :1024] + bq
  V[s,e]   = xT.T @ WvT
  scores[sq,sk] = QT.T @ KT  (per 128-row q-block, 512-wide sk groups)
  attn = exp(scores/32)           (unnormalized; scores ~ N(0,1), no
                                  max-subtraction needed; row-sums via
                                  activation accum_out)
  out[sq,e] = (attnT.T @ V) * (1/rowsum) + bv
"""
import numpy as np
import ml_dtypes

import concourse.bass as bass
import concourse.bacc as bacc
import concourse.mybir as mybir
from concourse.tile import TileContext
from concourse.bass_utils import run_bass_kernel_spmd
from concourse.masks import make_identity

B, S, E = 4, 2048, 1024
P = 128
EC = E // P          # 8 contraction chunks
FC = E // P          # 8 feature chunks
SKC = S // P         # 16 key chunks
SQ = S // 2          # queries per core
QB = SQ // P         # 8 query blocks per core
NG = 512             # moving-dim tile
INV_SCALE = 1.0 / float(np.sqrt(E))

MM_DT = mybir.dt.bfloat16
NP_MM = ml_dtypes.bfloat16
F32 = mybir.dt.float32

_CACHE = {}


def _build():
    nc = bacc.Bacc()
    xt = nc.declare_dram_parameter("xt", [E, S], MM_DT, isOutput=False)
    wqc = nc.declare_dram_parameter("wqc", [FC, P, EC, P], MM_DT, isOutput=False)
    wkc = nc.declare_dram_parameter("wkc", [FC, P, EC, P], MM_DT, isOutput=False)
    wvc = nc.declare_dram_parameter("wvc", [E // NG, P, EC, NG], MM_DT, isOutput=False)
    bqr = nc.declare_dram_parameter("bqr", [P, FC], F32, isOutput=False)
    bvb = nc.declare_dram_parameter("bvb", [P, E], F32, isOutput=False)
    out = nc.declare_dram_parameter("out", [SQ, E], F32, isOutput=True)

    xt_r = xt[:, :].rearrange("(ec p) s -> p ec s", p=P)

    with TileContext(nc) as tc:
        with (
            tc.tile_pool(name="wp", bufs=1) as wp,
            tc.tile_pool(name="kvq", bufs=1) as kvq,
            tc.tile_pool(name="att", bufs=2) as att_pool,
            tc.tile_pool(name="attT", bufs=2) as attT_pool,
            tc.tile_pool(name="outp", bufs=2) as outp,
            tc.tile_pool(name="smalls", bufs=2) as smalls,
            tc.tile_pool(name="ps", bufs=3, space="PSUM") as ps,
            tc.tile_pool(name="pstr", bufs=3, space="PSUM") as pstr,
        ):
            # ---- loads (ordered so the K projection can start earliest) ----
            ident = wp.tile([P, P], MM_DT)
            make_identity(nc, ident)

            w_sb = {}
            for name in ("wq", "wk", "wv"):
                w_sb[name] = wp.tile([P, EC, E], MM_DT, name=f"{name}_sb")
            xt_sb = wp.tile([P, EC, S], MM_DT)

            def load_xt_group(g):
                for ec in range(EC):
                    nc.sync.dma_start(
                        xt_sb[:, ec, g * NG:(g + 1) * NG],
                        xt_r[:, ec, g * NG:(g + 1) * NG],
                    )

            nc.sync.dma_start(w_sb["wk"][:, :, 0:P], wkc[0])
            load_xt_group(0)
            for fc in range(1, FC):
                nc.sync.dma_start(w_sb["wk"][:, :, fc * P:(fc + 1) * P], wkc[fc])
            load_xt_group(1)
            load_xt_group(2)
            load_xt_group(3)
            for fc in range(FC):
                nc.sync.dma_start(w_sb["wq"][:, :, fc * P:(fc + 1) * P], wqc[fc])
            for g in range(E // NG):
                nc.sync.dma_start(w_sb["wv"][:, :, g * NG:(g + 1) * NG], wvc[g])
            bq_sb = wp.tile([P, FC], F32)
            nc.sync.dma_start(bq_sb[:], bqr[:, :])
            bv_sb = wp.tile([P, E], F32)
            nc.sync.dma_start(bv_sb[:], bvb[:, :])

            # PE warmup: cover the initial DMA latency and release the HAM
            # clock throttle before real matmuls arrive (~8us of transposes,
            # serialized by WAW on one PSUM tile; results unused).
            warm_ps = ps.tile([P, P], MM_DT, tag="pv", bufs=2)
            for _ in range(60):
                nc.tensor.transpose(warm_ps[:], ident[:], ident[:])

            KT = kvq.tile([P, FC, S], MM_DT)
            QT = kvq.tile([P, FC, SQ], MM_DT)
            V = kvq.tile([P, SKC, E], MM_DT)

            # ---- K projection (KT[f, sk]), g-major to match DMA stream ----
            for g in range(S // NG):
                for fc in range(FC):
                    pk = ps.tile([P, NG], F32, tag="mm")
                    for ec in range(EC):
                        nc.tensor.matmul(
                            pk[:],
                            w_sb["wk"][:, ec, fc * P:(fc + 1) * P],
                            xt_sb[:, ec, g * NG:(g + 1) * NG],
                            start=(ec == 0),
                            stop=(ec == EC - 1),
                        )
                    nc.scalar.copy(KT[:, fc, g * NG:(g + 1) * NG], pk[:])

            # ---- Q projection (QT[f, sq] + bq) ----
            for fc in range(FC):
                for g in range(SQ // NG):
                    pq = ps.tile([P, NG], F32, tag="mm")
                    for ec in range(EC):
                        nc.tensor.matmul(
                            pq[:],
                            w_sb["wq"][:, ec, fc * P:(fc + 1) * P],
                            xt_sb[:, ec, g * NG:(g + 1) * NG],
                            start=(ec == 0),
                            stop=(ec == EC - 1),
                        )
                    nc.scalar.activation(
                        QT[:, fc, g * NG:(g + 1) * NG],
                        pq[:],
                        mybir.ActivationFunctionType.Identity,
                        bias=bq_sb[:, fc:fc + 1],
                    )

            # ---- V projection (V[sk, e]) ----
            for skc in range(SKC):
                for g in range(E // NG):
                    pv = ps.tile([P, NG], F32, tag="mm")
                    for ec in range(EC):
                        nc.tensor.matmul(
                            pv[:],
                            xt_sb[:, ec, skc * P:(skc + 1) * P],
                            w_sb["wv"][:, ec, g * NG:(g + 1) * NG],
                            start=(ec == 0),
                            stop=(ec == EC - 1),
                        )
                    nc.vector.tensor_copy(V[:, skc, g * NG:(g + 1) * NG], pv[:])

            # ---- attention per q-block ----
            for qb in range(QB):
                qsl = slice(qb * P, (qb + 1) * P)
                attn = att_pool.tile([P, S], MM_DT, tag="attn")
                sums4 = smalls.tile([P, S // NG], F32, tag="s4")
                for g in range(S // NG):
                    pscr = ps.tile([P, NG], F32, tag="mm")
                    for fc in range(FC):
                        nc.tensor.matmul(
                            pscr[:],
                            QT[:, fc, qsl],
                            KT[:, fc, g * NG:(g + 1) * NG],
                            start=(fc == 0),
                            stop=(fc == FC - 1),
                        )
                    nc.scalar.activation(
                        attn[:, g * NG:(g + 1) * NG],
                        pscr[:],
                        mybir.ActivationFunctionType.Exp,
                        scale=float(INV_SCALE),
                        accum_out=sums4[:, g:g + 1],
                    )
                ssum = smalls.tile([P, 1], F32, tag="ssum")
                nc.vector.reduce_sum(ssum[:], sums4[:], axis=mybir.AxisListType.X)
                recip = smalls.tile([P, 1], F32, tag="recip")
                nc.vector.reciprocal(recip[:], ssum[:])

                attT = attT_pool.tile([P, SKC, P], MM_DT, tag="attT")
                for skc in range(SKC):
                    pt = pstr.tile([P, P], MM_DT, tag="tr")
                    nc.tensor.transpose(pt[:], attn[:, skc * P:(skc + 1) * P], ident[:])
                    nc.vector.tensor_copy(attT[:, skc], pt[:])

                outt = outp.tile([P, E], F32, tag="out")
                for g in range(E // NG):
                    ppv = ps.tile([P, NG], F32, tag="pv", bufs=2)
                    for skc in range(SKC):
                        nc.tensor.matmul(
                            ppv[:],
                            attT[:, skc],
                            V[:, skc, g * NG:(g + 1) * NG],
                            start=(skc == 0),
                            stop=(skc == SKC - 1),
                        )
                    nc.scalar.activation(
                        outt[:, g * NG:(g + 1) * NG],
                        ppv[:],
                        mybir.ActivationFunctionType.Copy,
                        scale=recip[:, 0:1],
                    )
                    nc.vector.tensor_add(
                        outt[:, g * NG:(g + 1) * NG],
                        outt[:, g * NG:(g + 1) * NG],
                        bv_sb[:, g * NG:(g + 1) * NG],
                    )
                    nc.sync.dma_start(
                        out[qb * P:(qb + 1) * P, g * NG:(g + 1) * NG],
                        outt[:, g * NG:(g + 1) * NG],
                    )
    nc.finalize()
    return nc


def build_in_maps(x, Wq, bq, Wk, bk, Wv, bv):
    x = np.asarray(x, dtype=np.float32)

    def colchunk(W, n):
        # W.T is [E(e), E(f)]; -> [E//n(fchunk), P(p of e), EC(ec), n]
        wt = np.ascontiguousarray(np.asarray(W, np.float32).T).astype(NP_MM)
        return np.ascontiguousarray(
            wt.reshape(EC, P, E // n, n).transpose(2, 1, 0, 3)
        )

    wqc = colchunk(Wq, P)
    wkc = colchunk(Wk, P)
    wvc = colchunk(Wv, NG)
    bqr = np.ascontiguousarray(
        np.asarray(bq, np.float32).reshape(FC, P).T
    )  # [P, FC]; column fc = bq[fc*128:(fc+1)*128]
    bvb = np.broadcast_to(np.asarray(bv, np.float32)[None, :], (P, E)).copy()

    in_maps = []
    for c in range(8):
        b, h = divmod(c, 2)
        xt_full = np.ascontiguousarray(x[b].T).astype(NP_MM)  # [E, S]
        if h == 0:
            xt_perm = xt_full
        else:
            xt_perm = np.ascontiguousarray(
                np.concatenate([xt_full[:, SQ:], xt_full[:, :SQ]], axis=1)
            )
        in_maps.append(
            dict(xt=xt_perm, wqc=wqc, wkc=wkc, wvc=wvc, bqr=bqr, bvb=bvb)
        )

    return in_maps


def kernel(x, Wq, bq, Wk, bk, Wv, bv):
    if "nc" not in _CACHE:
        _CACHE["nc"] = _build()
    nc = _CACHE["nc"]
    in_maps = build_in_maps(x, Wq, bq, Wk, bk, Wv, bv)
    res = run_bass_kernel_spmd(nc, in_maps, list(range(8)))

    out = np.empty((B, S, E), np.float32)
    for c in range(8):
        b, h = divmod(c, 2)
        out[b, h * SQ:(h + 1) * SQ, :] = res.results[c]["out"]
    return out
